# revision 20
# baseline (speedup 1.0000x reference)
"""Trainium2 Bass kernel for the EnforcedNeuralODE recurrence.

Reference computation (per timestep):
    x_t = fc_w @ concat(x_{t-1}, f_{t-1}) + fc_b
      i.e. x_t = Wx x_{t-1} + Wf f_{t-1} + b
over T-1 = 4095 steps, batch 256, state 64, force 64.
Output: [T, B, 64] = concat([x_0], [x_1..x_{T-1}]).

Strategy: data-parallel batch shard (32 samples/core across 8 cores); on
each core a blocked parallel scan over K=32-step blocks.  All matmuls
contract over the full 128 partitions and pack the two parities of a
step-pair into the two PSUM partition halves (even steps rows 0:64, odd
rows 64:128), so every PE stream does maximal work per column:

  P1: pair chain producing BOTH parities of the within-block prefix
      (bias included), batched across the chunk's blocks in the free dim:
        [h_2p; h_2p+1] = W1a.[f_2p; f_2p+1] + W1b.[.; h_2p-1]  (+bias)
      where W1a = [[Wf, 0], [WxWf, Wf]], W1b = [[0, Wx], [0, Wx^2]]
      (block layout; lhsT stored transposed), 2 matmuls per 2 steps.
  P2: block-boundary scan s_{b+1} = Wx^K s_b + h_{K-1}: two small
      128-row matmuls per block.
  P3: combine: one matmul  [Wx^{2p+1}; Wx^{2p+2}] . s  per pair, then a
      VectorE tensor_add of the prefix h-pair (PSUM + SBUF -> SBUF bf16),
      then DMA out (bf16 halves the output HBM traffic; host casts f32).

Software pipelining: chunk c's P1 pair-loop interleaves chunk c-1's P2
scan steps (first half of the loop) and chunk c-1's P3 combine streams
(second half), so the in-order tensor queue always holds independent
stream work between the chain's scalar-evacuation waits.

Hardware notes:
  - Every matmul reads operand partitions 0..128 (unused halves are
    host-zeroed), avoiding the mixed-operand-partition-half PE crash
    (NRT_EXEC_UNIT_UNRECOVERABLE) seen when accumulating matmuls whose
    operands sit on different halves.
  - bf16 operands run the PE at 1 row/cycle; PSUM accumulation fp32.
"""

import numpy as np
from contextlib import ExitStack

NCORES = 8
BATCH, STATE, FDIM, TIMESPAN = 256, 64, 64, 4096

# per-core tiling
BC = BATCH // NCORES        # 32 batch per core
K = 32                      # steps per block
PAIRS = K // 2              # 16
NB = TIMESPAN // K          # 128 blocks (steps padded 4095 -> 4096)
NBC = 16                    # blocks per chunk
CHUNKS = NB // NBC          # 8
N = NBC * BC                # 512 free-dim per step column
F_COLS = PAIRS * N          # 8192 forcing cols per chunk (parity-stacked)
H_COLS = PAIRS * N          # 8192 prefix cols per chunk
O_COLS = PAIRS * N          # 8192 output cols per chunk (pair-packed)

_NC_CACHE: dict = {}

MM_DTYPE = "bfloat16"
OUT_DTYPE = "bfloat16"


def _set_dims(ncores=8, bc=32, k=32, nbc=16, chunks=8):
    """Override problem dims (testing only). Recomputes derived globals."""
    global NCORES, BATCH, BC, K, PAIRS, NB, NBC, CHUNKS, N
    global F_COLS, H_COLS, O_COLS, TIMESPAN
    NCORES, BC, K, NBC, CHUNKS = ncores, bc, k, nbc, chunks
    BATCH = NCORES * BC
    PAIRS = K // 2
    NB = CHUNKS * NBC
    TIMESPAN = NB * K
    N = NBC * BC
    F_COLS = PAIRS * N
    H_COLS = PAIRS * N
    O_COLS = PAIRS * N


def _build_nc(chunks, nbc, bc, k):
    """Build + compile the per-core Bass module (SPMD: same NEFF all cores)."""
    import concourse.bass as bass  # noqa: F401
    import concourse.tile as tile
    from concourse import bacc, mybir

    pairs = k // 2
    n = nbc * bc
    f_cols = pairs * n
    h_cols = pairs * n
    o_cols = pairs * n
    nb = chunks * nbc
    f32 = mybir.dt.float32
    mdt = getattr(mybir.dt, MM_DTYPE)
    odt = getattr(mybir.dt, OUT_DTYPE)
    AF = mybir.ActivationFunctionType

    nc = bacc.Bacc("TRN2", target_bir_lowering=False, debug=False)

    f_dram = nc.dram_tensor("f", [128, chunks * f_cols], mdt, kind="ExternalInput")
    w1_dram = nc.dram_tensor("w1", [128, 256], mdt, kind="ExternalInput")
    wpow_dram = nc.dram_tensor("wpow", [128, k * 64], mdt, kind="ExternalInput")
    wp2_dram = nc.dram_tensor("wp2", [128, 256], mdt, kind="ExternalInput")
    bias_dram = nc.dram_tensor("bias", [128, 1], mdt, kind="ExternalInput")
    s0_dram = nc.dram_tensor("s0", [128, (nb + 1) * bc], mdt, kind="ExternalInput")
    out_dram = nc.dram_tensor("out", [128, chunks * o_cols], odt, kind="ExternalOutput")

    with tile.TileContext(nc) as tc, ExitStack() as ctx:
        singles = ctx.enter_context(tc.tile_pool(name="singles", bufs=1))
        fpool = ctx.enter_context(tc.tile_pool(name="fpool", bufs=4))
        hpool = ctx.enter_context(tc.tile_pool(name="hpool", bufs=5))
        opool = ctx.enter_context(tc.tile_pool(name="opool", bufs=4))
        p1ps = ctx.enter_context(tc.tile_pool(name="p1ps", bufs=4, space="PSUM"))
        p3ps = ctx.enter_context(tc.tile_pool(name="p3ps", bufs=3, space="PSUM"))
        p2ps = ctx.enter_context(tc.tile_pool(name="p2ps", bufs=1, space="PSUM"))

        w1 = singles.tile([128, 256], mdt)
        nc.sync.dma_start(out=w1[:], in_=w1_dram[:])
        wpow = singles.tile([128, k * 64], mdt)
        nc.sync.dma_start(out=wpow[:], in_=wpow_dram[:])
        wp2 = singles.tile([128, 256], mdt)
        nc.sync.dma_start(out=wp2[:], in_=wp2_dram[:])
        bias = singles.tile([128, 1], mdt)
        nc.sync.dma_start(out=bias[:], in_=bias_dram[:])
        # block start states: rows 0:64 state (cols 0:bc = x0^T, rest
        # written by P2), rows 64:128 host-zeroed (matmuls read 0:128).
        s_t = singles.tile([128, (nb + 1) * bc], mdt)
        nc.sync.dma_start(out=s_t[:], in_=s0_dram[:])

        w1a = w1[:, 0:128]
        w1b = w1[:, 128:256]
        wp2m = wp2[:, 0:64]     # [Wx^K; I] merged scan lhsT
        wi128 = wp2[:, 64:192]  # I128 for P3 tail-mode h accumulate

        os_pairs = pairs // 2
        os_cols = os_pairs * n

        def emit_hlast_copy(bg, htile):
            """GpSimd stages h_last(bg) into rows 64:128 of s-slot bg, so
            the scan step is ONE 128-row matmul with lhsT [Wx^K; I]."""
            blk = bg % nbc
            nc.gpsimd.tensor_copy(
                s_t[64:128, bg * bc : (bg + 1) * bc],
                htile[64:128, (pairs - 1) * n + blk * bc : (pairs - 1) * n + (blk + 1) * bc],
            )

        def emit_p2_step(bg, on_scalar):
            """s_{bg+1} = [Wx^K; I] . [s_bg; h_last(bg)]: one matmul plus
            a PSUM->SBUF cast, alternating ScalarE/VectorE to balance."""
            ps2 = p2ps.tile([64, bc], f32, tag="PS2")
            nc.tensor.matmul(
                ps2[:], wp2m, s_t[:, bg * bc : (bg + 1) * bc],
                start=True, stop=True,
            )
            if on_scalar:
                nc.scalar.activation(
                    s_t[0:64, (bg + 1) * bc : (bg + 2) * bc], ps2[:], AF.Copy
                )
            else:
                nc.vector.tensor_copy(
                    s_t[0:64, (bg + 1) * bc : (bg + 2) * bc], ps2[:]
                )

        # per-chunk P3 state threaded through the interleaved emission
        class P3State:
            def __init__(self, c, htile, mode="add"):
                self.c = c
                self.htile = htile
                self.mode = mode
                self.ostage = None

            def emit_pair(self, p):
                ohalf, pp = divmod(p, os_pairs)
                if pp == 0:
                    self.ostage = opool.tile([128, os_cols], odt, tag="OS", name=f"os{self.c}_{p}")
                scol = s_t[:, self.c * n : (self.c + 1) * n]
                px = p3ps.tile([128, n], f32, tag="PX")
                if self.mode == "cast":
                    nc.tensor.matmul(
                        px[:], wpow[:, 2 * p * 64 : (2 * p + 2) * 64], scol,
                        start=True, stop=False,
                    )
                    nc.tensor.matmul(
                        px[:], wi128, self.htile[:, p * n : (p + 1) * n],
                        start=False, stop=True,
                    )
                    if p % 2 == 0:
                        nc.scalar.activation(
                            self.ostage[:, pp * n : (pp + 1) * n], px[:], AF.Copy
                        )
                    else:
                        nc.vector.tensor_copy(
                            self.ostage[:, pp * n : (pp + 1) * n], px[:]
                        )
                    if pp == os_pairs - 1:
                        nc.sync.dma_start(
                            out=out_dram[:, self.c * o_cols + ohalf * os_cols : self.c * o_cols + (ohalf + 1) * os_cols],
                            in_=self.ostage[:],
                        )
                    return
                nc.tensor.matmul(
                    px[:], wpow[:, 2 * p * 64 : (2 * p + 2) * 64], scol,
                    start=True, stop=True,
                )
                nc.vector.tensor_add(
                    self.ostage[:, pp * n : (pp + 1) * n], px[:],
                    self.htile[:, p * n : (p + 1) * n],
                )
                if pp == os_pairs - 1:
                    nc.sync.dma_start(
                        out=out_dram[:, self.c * o_cols + ohalf * os_cols : self.c * o_cols + (ohalf + 1) * os_cols],
                        in_=self.ostage[:],
                    )

        # Two chunks' P1 chains advance in lockstep (A/B): while chain A
        # waits on its scalar evacuation, chain B's streams keep the PE
        # busy so the tensor activity window stays warm (2.4 GHz).  The
        # PREVIOUS group's P2 scan steps (2/iter, one between the a-pair
        # and one between the b-pair so >=2 independent streams cover
        # each chain wait) and P3 combines (chunk g-2 in iters 8..11;
        # chunk g-1 as an end-of-group burst once its scan is done) are
        # interleaved as backlog.
        assert chunks % 2 == 0 and pairs == nbc
        # ~3.5us of dependency-free junk matmuls while the first F tile
        # DMA lands: the PE activity window warms to 2.4 GHz for free
        for j in range(48):
            jp = p2ps.tile([64, bc], f32, tag="PS2", name="junk")
            nc.tensor.matmul(jp[:], wp2m, s_t[:, 0:bc], start=True, stop=True)

        htiles: dict = {}
        for g in range(0, chunks, 2):
            ftl = {}
            for c in (g, g + 1):
                ftl[c] = fpool.tile([128, f_cols], mdt, tag="F", name=f"ft{c}")
                nsplit = 4 if c == 0 else 2
                for fh in range(nsplit):
                    fw = f_cols // nsplit
                    nc.sync.dma_start(
                        out=ftl[c][:, fh * fw : (fh + 1) * fw],
                        in_=f_dram[:, c * f_cols + fh * fw : c * f_cols + (fh + 1) * fw],
                    )
                htiles[c] = hpool.tile([128, h_cols], mdt, tag="H", name=f"ht{c}")
            if g > 0:
                for bg in range((g - 2) * nbc, g * nbc):
                    emit_hlast_copy(bg, htiles[bg // nbc])
                bg0 = (g - 2) * nbc
            p3_first = P3State(g - 3, htiles[g - 3]) if g >= 4 else None
            p3_second = P3State(g - 2, htiles[g - 2]) if g >= 2 else None

            for p in range(pairs):
                # chain step 2p: cast lands at the FRONT of its engine's
                # FIFO this iteration, so the serial scan never queues
                # behind bulk evacuation work
                if g > 0:
                    emit_p2_step(bg0 + 2 * p, on_scalar=(p % 2 == 0))
                pss = {}
                for c in (g, g + 1):  # a-matmuls adjacent: shared w1a load
                    pss[c] = p1ps.tile([128, n], f32, tag="PS1", name=f"ps1_{c % 2}")
                    nc.tensor.matmul(
                        pss[c][:], w1a, ftl[c][:, p * n : (p + 1) * n],
                        start=True, stop=(p == 0),
                    )
                if g > 0:
                    emit_p2_step(bg0 + 2 * p + 1, on_scalar=(p % 2 == 1))
                if p > 0:
                    for c in (g, g + 1):  # b-matmuls adjacent: shared w1b
                        nc.tensor.matmul(
                            pss[c][:], w1b, htiles[c][:, (p - 1) * n : p * n],
                            start=False, stop=True,
                        )
                for c in (g, g + 1):
                    nc.scalar.activation(
                        htiles[c][:, p * n : (p + 1) * n], pss[c][:],
                        AF.Identity, bias=bias[:, 0:1],
                    )
                # combine backlog 2/iter throughout: chunk g-3 (scan long
                # done) in the first half, chunk g-2 (scan finishes at
                # p~7) in the second half
                if p < pairs // 2:
                    if p3_first is not None:
                        p3_first.emit_pair(2 * p)
                        p3_first.emit_pair(2 * p + 1)
                elif p3_second is not None:
                    p3_second.emit_pair(2 * (p - pairs // 2))
                    p3_second.emit_pair(2 * (p - pairs // 2) + 1)
            for c in (g - 3, g - 2):
                if c >= 0:
                    del htiles[c]

        # epilogue: chunks-3 combine, then the last two chunks' scan
        # chains interleaved with combines as they unblock
        last = chunks - 1
        for bg in range((chunks - 2) * nbc, chunks * nbc):
            emit_hlast_copy(bg, htiles[bg // nbc])
        p3a = P3State(chunks - 3, htiles[chunks - 3])
        for blk in range(nbc):
            emit_p2_step((chunks - 2) * nbc + blk, on_scalar=True)
            p3a.emit_pair(blk)
        p3b = P3State(chunks - 2, htiles[chunks - 2])
        for blk in range(nbc):
            emit_p2_step((chunks - 1) * nbc + blk, on_scalar=True)
            p3b.emit_pair(blk)
        p3c = P3State(last, htiles[last], mode="cast")
        for pr in range(pairs):
            p3c.emit_pair(pr)

    nc.compile()
    return nc


def _get_nc():
    key = (CHUNKS, NBC, BC, K)
    if key not in _NC_CACHE:
        _NC_CACHE[key] = _build_nc(CHUNKS, NBC, BC, K)
    return _NC_CACHE[key]


def _host_prep(inputs, forcing, fc_w, fc_b):
    """Build per-core input maps (numpy only, untimed)."""
    S = STATE
    fc_w = np.asarray(fc_w, np.float32)
    fc_b = np.asarray(fc_b, np.float32)
    Wx = fc_w[:, :S].astype(np.float64)
    Wf = fc_w[:, S:].astype(np.float64)
    b = fc_b.astype(np.float64)

    if MM_DTYPE == "bfloat16":
        import ml_dtypes

        mm_np = ml_dtypes.bfloat16
    else:
        mm_np = np.float32

    # w1: [w1a | w1b] lhsT blocks (out halves: rows 0:64 even, 64:128 odd)
    w1 = np.zeros((128, 256), np.float64)
    w1[0:64, 0:64] = Wf.T            # f_even -> even
    w1[0:64, 64:128] = (Wx @ Wf).T   # f_even -> odd
    w1[64:128, 64:128] = Wf.T        # f_odd  -> odd
    w1[64:128, 128:192] = Wx.T       # h_prev -> even
    w1[64:128, 192:256] = (Wx @ Wx).T  # h_prev -> odd

    # wpow: col block j holds (Wx^{j+1})^T in rows 0:64; rows 64:128 zero
    wpow = np.zeros((128, K * 64), np.float64)
    P = np.eye(S, dtype=np.float64)
    for j in range(K):
        P = Wx @ P
        wpow[0:64, j * 64 : (j + 1) * 64] = P.T
        if j == K - 1:
            wxk = P

    # wp2 col block 0: merged scan lhsT [Wx^K; I] (contraction over
    # [s_b; h_last] stacked in the s-tile's partition halves)
    wp2 = np.zeros((128, 256), np.float64)
    wp2[0:64, 0:64] = wxk.T
    wp2[64:128, 0:64] = np.eye(64)
    wp2[:, 64:192] = np.eye(128)  # I128: P3 tail-mode h accumulate

    bias128 = np.zeros((128, 1), np.float64)
    bias128[0:64, 0] = b
    bias128[64:128, 0] = Wx @ b + b

    # forcing: [T-1, B, F] -> pad -> [parity*feat, c, p, blk, bfull]
    steps = TIMESPAN
    fpad = np.zeros((steps, BATCH, FDIM), np.float32)
    fpad[: TIMESPAN - 1] = np.asarray(forcing, np.float32)
    # t = (c*NBC + blk)*K + 2p + parity
    arr = fpad.reshape(CHUNKS, NBC, PAIRS, 2, BATCH, FDIM)
    arr = arr.transpose(3, 5, 0, 2, 1, 4)  # [parity, feat, c, p, blk, bfull]

    inputs = np.asarray(inputs, np.float32)
    w1 = w1.astype(mm_np)
    wpow = wpow.astype(mm_np)
    wp2 = wp2.astype(mm_np)
    bias128 = bias128.astype(mm_np)
    in_maps = []
    for core in range(NCORES):
        bs = slice(core * BC, (core + 1) * BC)
        fcore = (
            np.ascontiguousarray(arr[..., bs])
            .reshape(128, CHUNKS * F_COLS)
            .astype(mm_np)
        )
        s0 = np.zeros((128, (NB + 1) * BC), mm_np)
        s0[0:64, 0:BC] = inputs[bs].T.astype(mm_np)
        in_maps.append(
            {
                "f": fcore,
                "w1": w1,
                "wpow": wpow,
                "wp2": wp2,
                "bias": bias128,
                "s0": s0,
            }
        )
    return in_maps


def _host_decode(results, inputs):
    """Per-core out [128, CHUNKS*O_COLS] -> full [T, B, S]."""
    inputs = np.asarray(inputs, np.float32)
    out = np.empty((TIMESPAN, BATCH, STATE), np.float32)
    out[0] = inputs
    for core in range(NCORES):
        o = results[core]["out"].astype(np.float32)
        o = o.reshape(2, 64, CHUNKS, PAIRS, NBC, BC)
        # [parity, s, c, p, blk, b] -> [c, blk, p, parity, b, s]
        o = o.transpose(2, 4, 3, 0, 5, 1).reshape(TIMESPAN, BC, STATE)
        out[1:, core * BC : (core + 1) * BC] = o[: TIMESPAN - 1]
    return out


def kernel(inputs, forcing, fc_w, fc_b, timespan):
    from concourse.bass_utils import run_bass_kernel_spmd

    timespan = int(timespan)
    assert timespan == TIMESPAN, f"hardcoded for timespan={TIMESPAN}, got {timespan}"
    nc = _get_nc()
    in_maps = _host_prep(inputs, forcing, fc_w, fc_b)
    res = run_bass_kernel_spmd(nc, in_maps, core_ids=list(range(NCORES)))
    return _host_decode(res.results, inputs)


if __name__ == "__main__":
    nc = _get_nc()
    print("built ok")


# revision 21
# speedup vs baseline: 1.0145x; 1.0145x over previous
"""Trainium2 Bass kernel for the EnforcedNeuralODE recurrence.

Reference computation (per timestep):
    x_t = fc_w @ concat(x_{t-1}, f_{t-1}) + fc_b
      i.e. x_t = Wx x_{t-1} + Wf f_{t-1} + b
over T-1 = 4095 steps, batch 256, state 64, force 64.
Output: [T, B, 64] = concat([x_0], [x_1..x_{T-1}]).

Strategy: data-parallel batch shard (32 samples/core across 8 cores); on
each core a blocked parallel scan over K=32-step blocks.  All matmuls
contract over the full 128 partitions and pack the two parities of a
step-pair into the two PSUM partition halves (even steps rows 0:64, odd
rows 64:128), so every PE stream does maximal work per column:

  P1: pair chain producing BOTH parities of the within-block prefix
      (bias included), batched across the chunk's blocks in the free dim:
        [h_2p; h_2p+1] = W1a.[f_2p; f_2p+1] + W1b.[.; h_2p-1]  (+bias)
      where W1a = [[Wf, 0], [WxWf, Wf]], W1b = [[0, Wx], [0, Wx^2]]
      (block layout; lhsT stored transposed), 2 matmuls per 2 steps.
  P2: block-boundary scan s_{b+1} = Wx^K s_b + h_{K-1}: two small
      128-row matmuls per block.
  P3: combine: one matmul  [Wx^{2p+1}; Wx^{2p+2}] . s  per pair, then a
      VectorE tensor_add of the prefix h-pair (PSUM + SBUF -> SBUF bf16),
      then DMA out (bf16 halves the output HBM traffic; host casts f32).

Software pipelining: chunk c's P1 pair-loop interleaves chunk c-1's P2
scan steps (first half of the loop) and chunk c-1's P3 combine streams
(second half), so the in-order tensor queue always holds independent
stream work between the chain's scalar-evacuation waits.

Hardware notes:
  - Every matmul reads operand partitions 0..128 (unused halves are
    host-zeroed), avoiding the mixed-operand-partition-half PE crash
    (NRT_EXEC_UNIT_UNRECOVERABLE) seen when accumulating matmuls whose
    operands sit on different halves.
  - bf16 operands run the PE at 1 row/cycle; PSUM accumulation fp32.
"""

import numpy as np
from contextlib import ExitStack

NCORES = 8
BATCH, STATE, FDIM, TIMESPAN = 256, 64, 64, 4096

# per-core tiling
BC = BATCH // NCORES        # 32 batch per core
K = 32                      # steps per block
PAIRS = K // 2              # 16
NB = TIMESPAN // K          # 128 blocks (steps padded 4095 -> 4096)
NBC = 16                    # blocks per chunk
CHUNKS = NB // NBC          # 8
N = NBC * BC                # 512 free-dim per step column
F_COLS = PAIRS * N          # 8192 forcing cols per chunk (parity-stacked)
H_COLS = PAIRS * N          # 8192 prefix cols per chunk
O_COLS = PAIRS * N          # 8192 output cols per chunk (pair-packed)

_NC_CACHE: dict = {}

MM_DTYPE = "bfloat16"
OUT_DTYPE = "bfloat16"


def _set_dims(ncores=8, bc=32, k=32, nbc=16, chunks=8):
    """Override problem dims (testing only). Recomputes derived globals."""
    global NCORES, BATCH, BC, K, PAIRS, NB, NBC, CHUNKS, N
    global F_COLS, H_COLS, O_COLS, TIMESPAN
    NCORES, BC, K, NBC, CHUNKS = ncores, bc, k, nbc, chunks
    BATCH = NCORES * BC
    PAIRS = K // 2
    NB = CHUNKS * NBC
    TIMESPAN = NB * K
    N = NBC * BC
    F_COLS = PAIRS * N
    H_COLS = PAIRS * N
    O_COLS = PAIRS * N


def _build_nc(chunks, nbc, bc, k):
    """Build + compile the per-core Bass module (SPMD: same NEFF all cores)."""
    import concourse.bass as bass  # noqa: F401
    import concourse.tile as tile
    from concourse import bacc, mybir

    pairs = k // 2
    n = nbc * bc
    f_cols = pairs * n
    h_cols = pairs * n
    o_cols = pairs * n
    nb = chunks * nbc
    f32 = mybir.dt.float32
    mdt = getattr(mybir.dt, MM_DTYPE)
    odt = getattr(mybir.dt, OUT_DTYPE)
    AF = mybir.ActivationFunctionType

    nc = bacc.Bacc("TRN2", target_bir_lowering=False, debug=False)

    f_dram = nc.dram_tensor("f", [128, chunks * f_cols], mdt, kind="ExternalInput")
    w1_dram = nc.dram_tensor("w1", [128, 256], mdt, kind="ExternalInput")
    wpow_dram = nc.dram_tensor("wpow", [128, k * 64], mdt, kind="ExternalInput")
    wp2_dram = nc.dram_tensor("wp2", [128, 256], mdt, kind="ExternalInput")
    bias_dram = nc.dram_tensor("bias", [128, 1], mdt, kind="ExternalInput")
    s0_dram = nc.dram_tensor("s0", [128, (nb + 1) * bc], mdt, kind="ExternalInput")
    out_dram = nc.dram_tensor("out", [128, chunks * o_cols], odt, kind="ExternalOutput")

    with tile.TileContext(nc) as tc, ExitStack() as ctx:
        singles = ctx.enter_context(tc.tile_pool(name="singles", bufs=1))
        fpool = ctx.enter_context(tc.tile_pool(name="fpool", bufs=4))
        hpool = ctx.enter_context(tc.tile_pool(name="hpool", bufs=5))
        opool = ctx.enter_context(tc.tile_pool(name="opool", bufs=4))
        p1ps = ctx.enter_context(tc.tile_pool(name="p1ps", bufs=4, space="PSUM"))
        p3ps = ctx.enter_context(tc.tile_pool(name="p3ps", bufs=3, space="PSUM"))
        p2ps = ctx.enter_context(tc.tile_pool(name="p2ps", bufs=1, space="PSUM"))

        w1 = singles.tile([128, 256], mdt)
        nc.sync.dma_start(out=w1[:], in_=w1_dram[:])
        wpow = singles.tile([128, k * 64], mdt)
        nc.sync.dma_start(out=wpow[:], in_=wpow_dram[:])
        wp2 = singles.tile([128, 256], mdt)
        nc.sync.dma_start(out=wp2[:], in_=wp2_dram[:])
        bias = singles.tile([128, 1], mdt)
        nc.sync.dma_start(out=bias[:], in_=bias_dram[:])
        # block start states: rows 0:64 state (cols 0:bc = x0^T, rest
        # written by P2), rows 64:128 host-zeroed (matmuls read 0:128).
        s_t = singles.tile([128, (nb + 1) * bc], mdt)
        nc.sync.dma_start(out=s_t[:], in_=s0_dram[:])

        w1a = w1[:, 0:128]
        w1b = w1[:, 128:256]
        wp2m = wp2[:, 0:64]     # [Wx^K; I] merged scan lhsT
        wi128 = wp2[:, 64:192]  # I128 for P3 tail-mode h accumulate

        os_pairs = pairs // 2
        os_cols = os_pairs * n

        def emit_hlast_copy(bg, htile):
            """GpSimd stages h_last(bg) into rows 64:128 of s-slot bg, so
            the scan step is ONE 128-row matmul with lhsT [Wx^K; I]."""
            blk = bg % nbc
            nc.gpsimd.tensor_copy(
                s_t[64:128, bg * bc : (bg + 1) * bc],
                htile[64:128, (pairs - 1) * n + blk * bc : (pairs - 1) * n + (blk + 1) * bc],
            )

        def emit_p2_step(bg, on_scalar):
            """s_{bg+1} = [Wx^K; I] . [s_bg; h_last(bg)]: one matmul plus
            a PSUM->SBUF cast, alternating ScalarE/VectorE to balance."""
            ps2 = p2ps.tile([64, bc], f32, tag="PS2")
            nc.tensor.matmul(
                ps2[:], wp2m, s_t[:, bg * bc : (bg + 1) * bc],
                start=True, stop=True,
            )
            if on_scalar:
                nc.scalar.activation(
                    s_t[0:64, (bg + 1) * bc : (bg + 2) * bc], ps2[:], AF.Copy
                )
            else:
                nc.vector.tensor_copy(
                    s_t[0:64, (bg + 1) * bc : (bg + 2) * bc], ps2[:]
                )

        # per-chunk P3 state threaded through the interleaved emission
        class P3State:
            def __init__(self, c, htile, mode="add"):
                self.c = c
                self.htile = htile
                self.mode = mode
                self.ostage = None

            def emit_pair(self, p):
                ohalf, pp = divmod(p, os_pairs)
                if pp == 0:
                    self.ostage = opool.tile([128, os_cols], odt, tag="OS", name=f"os{self.c}_{p}")
                scol = s_t[:, self.c * n : (self.c + 1) * n]
                px = p3ps.tile([128, n], f32, tag="PX")
                if self.mode == "cast":
                    nc.tensor.matmul(
                        px[:], wpow[:, 2 * p * 64 : (2 * p + 2) * 64], scol,
                        start=True, stop=False,
                    )
                    nc.tensor.matmul(
                        px[:], wi128, self.htile[:, p * n : (p + 1) * n],
                        start=False, stop=True,
                    )
                    if p % 2 == 0:
                        nc.scalar.activation(
                            self.ostage[:, pp * n : (pp + 1) * n], px[:], AF.Copy
                        )
                    else:
                        nc.vector.tensor_copy(
                            self.ostage[:, pp * n : (pp + 1) * n], px[:]
                        )
                    if pp == os_pairs - 1:
                        nc.sync.dma_start(
                            out=out_dram[:, self.c * o_cols + ohalf * os_cols : self.c * o_cols + (ohalf + 1) * os_cols],
                            in_=self.ostage[:],
                        )
                    return
                nc.tensor.matmul(
                    px[:], wpow[:, 2 * p * 64 : (2 * p + 2) * 64], scol,
                    start=True, stop=True,
                )
                nc.vector.tensor_add(
                    self.ostage[:, pp * n : (pp + 1) * n], px[:],
                    self.htile[:, p * n : (p + 1) * n],
                )
                if pp == os_pairs - 1:
                    nc.sync.dma_start(
                        out=out_dram[:, self.c * o_cols + ohalf * os_cols : self.c * o_cols + (ohalf + 1) * os_cols],
                        in_=self.ostage[:],
                    )

        # Two chunks' P1 chains advance in lockstep (A/B): while chain A
        # waits on its scalar evacuation, chain B's streams keep the PE
        # busy so the tensor activity window stays warm (2.4 GHz).  The
        # PREVIOUS group's P2 scan steps (2/iter, one between the a-pair
        # and one between the b-pair so >=2 independent streams cover
        # each chain wait) and P3 combines (chunk g-2 in iters 8..11;
        # chunk g-1 as an end-of-group burst once its scan is done) are
        # interleaved as backlog.
        assert chunks % 2 == 0 and pairs == nbc
        htiles: dict = {}
        for g in range(0, chunks, 2):
            ftl = {}
            for c in (g, g + 1):
                ftl[c] = fpool.tile([128, f_cols], mdt, tag="F", name=f"ft{c}")
                for fh in range(2):
                    nc.sync.dma_start(
                        out=ftl[c][:, fh * (f_cols // 2) : (fh + 1) * (f_cols // 2)],
                        in_=f_dram[:, c * f_cols + fh * (f_cols // 2) : c * f_cols + (fh + 1) * (f_cols // 2)],
                    )
                htiles[c] = hpool.tile([128, h_cols], mdt, tag="H", name=f"ht{c}")
            if g > 0:
                for bg in range((g - 2) * nbc, g * nbc):
                    emit_hlast_copy(bg, htiles[bg // nbc])
                bg0 = (g - 2) * nbc
            p3_first = P3State(g - 3, htiles[g - 3]) if g >= 4 else None
            p3_second = P3State(g - 2, htiles[g - 2]) if g >= 2 else None

            for p in range(pairs):
                # chain step 2p: cast lands at the FRONT of its engine's
                # FIFO this iteration, so the serial scan never queues
                # behind bulk evacuation work
                if g > 0:
                    emit_p2_step(bg0 + 2 * p, on_scalar=(p % 2 == 0))
                pss = {}
                for c in (g, g + 1):  # a-matmuls adjacent: shared w1a load
                    pss[c] = p1ps.tile([128, n], f32, tag="PS1", name=f"ps1_{c % 2}")
                    nc.tensor.matmul(
                        pss[c][:], w1a, ftl[c][:, p * n : (p + 1) * n],
                        start=True, stop=(p == 0),
                    )
                if g > 0:
                    emit_p2_step(bg0 + 2 * p + 1, on_scalar=(p % 2 == 1))
                if p > 0:
                    for c in (g, g + 1):  # b-matmuls adjacent: shared w1b
                        nc.tensor.matmul(
                            pss[c][:], w1b, htiles[c][:, (p - 1) * n : p * n],
                            start=False, stop=True,
                        )
                for c in (g, g + 1):
                    nc.scalar.activation(
                        htiles[c][:, p * n : (p + 1) * n], pss[c][:],
                        AF.Identity, bias=bias[:, 0:1],
                    )
                # combine backlog 2/iter throughout: chunk g-3 (scan long
                # done) in the first half, chunk g-2 (scan finishes at
                # p~7) in the second half
                if p < pairs // 2:
                    if p3_first is not None:
                        p3_first.emit_pair(2 * p)
                        p3_first.emit_pair(2 * p + 1)
                elif p3_second is not None:
                    p3_second.emit_pair(2 * (p - pairs // 2))
                    p3_second.emit_pair(2 * (p - pairs // 2) + 1)
            for c in (g - 3, g - 2):
                if c >= 0:
                    del htiles[c]

        # epilogue: chunks-3 combine, then the last two chunks' scan
        # chains interleaved with combines as they unblock
        last = chunks - 1
        for bg in range((chunks - 2) * nbc, chunks * nbc):
            emit_hlast_copy(bg, htiles[bg // nbc])
        p3a = P3State(chunks - 3, htiles[chunks - 3])
        for blk in range(nbc):
            emit_p2_step((chunks - 2) * nbc + blk, on_scalar=(blk % 2 == 0))
            p3a.emit_pair(blk)
        p3b = P3State(chunks - 2, htiles[chunks - 2])
        for blk in range(nbc):
            emit_p2_step((chunks - 1) * nbc + blk, on_scalar=(blk % 2 == 0))
            p3b.emit_pair(blk)
        p3c = P3State(last, htiles[last], mode="cast")
        for pr in range(pairs):
            p3c.emit_pair(pr)

    nc.compile()
    return nc


def _get_nc():
    key = (CHUNKS, NBC, BC, K)
    if key not in _NC_CACHE:
        _NC_CACHE[key] = _build_nc(CHUNKS, NBC, BC, K)
    return _NC_CACHE[key]


def _host_prep(inputs, forcing, fc_w, fc_b):
    """Build per-core input maps (numpy only, untimed)."""
    S = STATE
    fc_w = np.asarray(fc_w, np.float32)
    fc_b = np.asarray(fc_b, np.float32)
    Wx = fc_w[:, :S].astype(np.float64)
    Wf = fc_w[:, S:].astype(np.float64)
    b = fc_b.astype(np.float64)

    if MM_DTYPE == "bfloat16":
        import ml_dtypes

        mm_np = ml_dtypes.bfloat16
    else:
        mm_np = np.float32

    # w1: [w1a | w1b] lhsT blocks (out halves: rows 0:64 even, 64:128 odd)
    w1 = np.zeros((128, 256), np.float64)
    w1[0:64, 0:64] = Wf.T            # f_even -> even
    w1[0:64, 64:128] = (Wx @ Wf).T   # f_even -> odd
    w1[64:128, 64:128] = Wf.T        # f_odd  -> odd
    w1[64:128, 128:192] = Wx.T       # h_prev -> even
    w1[64:128, 192:256] = (Wx @ Wx).T  # h_prev -> odd

    # wpow: col block j holds (Wx^{j+1})^T in rows 0:64; rows 64:128 zero
    wpow = np.zeros((128, K * 64), np.float64)
    P = np.eye(S, dtype=np.float64)
    for j in range(K):
        P = Wx @ P
        wpow[0:64, j * 64 : (j + 1) * 64] = P.T
        if j == K - 1:
            wxk = P

    # wp2 col block 0: merged scan lhsT [Wx^K; I] (contraction over
    # [s_b; h_last] stacked in the s-tile's partition halves)
    wp2 = np.zeros((128, 256), np.float64)
    wp2[0:64, 0:64] = wxk.T
    wp2[64:128, 0:64] = np.eye(64)
    wp2[:, 64:192] = np.eye(128)  # I128: P3 tail-mode h accumulate

    bias128 = np.zeros((128, 1), np.float64)
    bias128[0:64, 0] = b
    bias128[64:128, 0] = Wx @ b + b

    # forcing: [T-1, B, F] -> pad -> [parity*feat, c, p, blk, bfull]
    steps = TIMESPAN
    fpad = np.zeros((steps, BATCH, FDIM), np.float32)
    fpad[: TIMESPAN - 1] = np.asarray(forcing, np.float32)
    # t = (c*NBC + blk)*K + 2p + parity
    arr = fpad.reshape(CHUNKS, NBC, PAIRS, 2, BATCH, FDIM)
    arr = arr.transpose(3, 5, 0, 2, 1, 4)  # [parity, feat, c, p, blk, bfull]

    inputs = np.asarray(inputs, np.float32)
    w1 = w1.astype(mm_np)
    wpow = wpow.astype(mm_np)
    wp2 = wp2.astype(mm_np)
    bias128 = bias128.astype(mm_np)
    in_maps = []
    for core in range(NCORES):
        bs = slice(core * BC, (core + 1) * BC)
        fcore = (
            np.ascontiguousarray(arr[..., bs])
            .reshape(128, CHUNKS * F_COLS)
            .astype(mm_np)
        )
        s0 = np.zeros((128, (NB + 1) * BC), mm_np)
        s0[0:64, 0:BC] = inputs[bs].T.astype(mm_np)
        in_maps.append(
            {
                "f": fcore,
                "w1": w1,
                "wpow": wpow,
                "wp2": wp2,
                "bias": bias128,
                "s0": s0,
            }
        )
    return in_maps


def _host_decode(results, inputs):
    """Per-core out [128, CHUNKS*O_COLS] -> full [T, B, S]."""
    inputs = np.asarray(inputs, np.float32)
    out = np.empty((TIMESPAN, BATCH, STATE), np.float32)
    out[0] = inputs
    for core in range(NCORES):
        o = results[core]["out"].astype(np.float32)
        o = o.reshape(2, 64, CHUNKS, PAIRS, NBC, BC)
        # [parity, s, c, p, blk, b] -> [c, blk, p, parity, b, s]
        o = o.transpose(2, 4, 3, 0, 5, 1).reshape(TIMESPAN, BC, STATE)
        out[1:, core * BC : (core + 1) * BC] = o[: TIMESPAN - 1]
    return out


def kernel(inputs, forcing, fc_w, fc_b, timespan):
    from concourse.bass_utils import run_bass_kernel_spmd

    timespan = int(timespan)
    assert timespan == TIMESPAN, f"hardcoded for timespan={TIMESPAN}, got {timespan}"
    nc = _get_nc()
    in_maps = _host_prep(inputs, forcing, fc_w, fc_b)
    res = run_bass_kernel_spmd(nc, in_maps, core_ids=list(range(NCORES)))
    return _host_decode(res.results, inputs)


if __name__ == "__main__":
    nc = _get_nc()
    print("built ok")
